# revision 1
# baseline (speedup 1.0000x reference)
"""Trainium2 Bass kernel for multi-head attention (B=8, N=1024, C=768, H=12).

Strategy: data-parallel over batch — core b computes batch element b entirely
locally (no collectives). Host prepares transposed bf16 inputs; device does
QKV^T, scores S[k,q] (softmax without max-subtraction — scores ~N(0,1), exp is
safe), exp on ACT directly from PSUM, attn@V with a ones-column for the
softmax denominators, reciprocal + PE-broadcast division, output projection.
"""

import numpy as np
import ml_dtypes

B, N, C = 8, 1024, 768
H, HD = 12, 64
SCALE = HD ** -0.5
CT = C // 128   # 6 c-tiles
NT = N // 128   # 8 seq tiles
QB = 2          # q blocks of 512
PAIRS = H // 2  # 6 head pairs


def build_nc():
    import concourse.bass as bass
    import concourse.mybir as mybir
    import concourse.tile as tile
    from concourse import bacc
    from contextlib import ExitStack

    BF = mybir.dt.bfloat16
    F32 = mybir.dt.float32
    F32R = mybir.dt.float32r
    EXP = mybir.ActivationFunctionType.Exp

    nc = bacc.Bacc()
    xT = nc.declare_dram_parameter("xT", [C, N], BF, isOutput=False)
    wqkA = nc.declare_dram_parameter("wqkA", [C, 512], BF, isOutput=False)
    wqkB = nc.declare_dram_parameter("wqkB", [C, 1024], BF, isOutput=False)
    wvT = nc.declare_dram_parameter("wvT", [C, C], BF, isOutput=False)
    wpT = nc.declare_dram_parameter("wpT", [C, C], BF, isOutput=False)
    sel = nc.declare_dram_parameter("sel", [12, H * 64], BF, isOutput=False)
    out = nc.declare_dram_parameter("out", [N, C], F32, isOutput=True)

    with tile.TileContext(nc, pool_alloc_mode="queue") as tc, ExitStack() as ctx:
        sb = ctx.enter_context(tc.tile_pool(name="sb", bufs=1))
        ptp = ctx.enter_context(tc.tile_pool(name="pt", bufs=22))
        attp = ctx.enter_context(tc.tile_pool(name="attp", bufs=2))
        yp = ctx.enter_context(tc.tile_pool(name="y", bufs=2))
        pbig = ctx.enter_context(tc.tile_pool(name="pbig", bufs=3, space="PSUM"))
        psmall = ctx.enter_context(tc.tile_pool(name="psmall", bufs=2, space="PSUM"))

        # ---- persistent SBUF tiles
        xT_sb = sb.tile([128, CT * N], BF, tag="xT")            # ct at cols ct*1024
        wqk_sb = sb.tile([128, CT * 1536], BF, tag="wqk")       # ct at cols ct*1536
        wv_sb = sb.tile([128, CT * C], BF, tag="wv")            # ct at cols ct*768
        wp_sb = sb.tile([128, CT * C], BF, tag="wp")
        qkT_sb = sb.tile([128, 12 * N], BF, tag="qkT")          # m-tile mt at cols mt*1024
        ves_sb = sb.tile([128, NT * 13 * 65], BF, tag="ves")    # kt at cols kt*845; head h at +h*65, ones col at +64; pad head slot 12
        attf_sb = sb.tile([128, CT * N], BF, tag="attf")        # divided attn output, c-major layout
        den_sb = sb.tile([12, N], BF, tag="den")
        denf_sb = sb.tile([12, N], F32, tag="denf")
        recip_sb = sb.tile([12, N], F32, tag="recip")
        rb_sb = sb.tile([12, N], BF, tag="rb")                  # bf16 recip for PE broadcast
        sel_sb = sb.tile([12, H * 64], BF, tag="sel")           # one-hot head selectors for PE broadcast of recip

        VS = 13 * 65  # 845 cols per kt block in ves (12 heads x 65 + padding)

        # ---- DMA inputs (xT/wqk interleaved per c-tile so the first QKV
        # accumulation step can start after ~2 DMAs, not 12)
        for ct in range(CT):
            nc.sync.dma_start(out=xT_sb[:, ct * N:(ct + 1) * N], in_=xT[ct * 128:(ct + 1) * 128, :])
            nc.sync.dma_start(out=wqk_sb[:, ct * 1536: ct * 1536 + 512], in_=wqkA[ct * 128:(ct + 1) * 128, :])
        for ct in range(CT):
            nc.sync.dma_start(out=wqk_sb[:, ct * 1536 + 512:(ct + 1) * 1536], in_=wqkB[ct * 128:(ct + 1) * 128, :])
        for ct in range(CT):
            nc.sync.dma_start(out=wv_sb[:, ct * C:(ct + 1) * C], in_=wvT[ct * 128:(ct + 1) * 128, :])
        for ct in range(CT):
            nc.sync.dma_start(out=wp_sb[:, ct * C:(ct + 1) * C], in_=wpT[ct * 128:(ct + 1) * 128, :])

        nc.sync.dma_start(out=sel_sb[:, :], in_=sel[:, :])
        warm_sb = sb.tile([1, 16], F32, tag="warm")
        nc.gpsimd.memset(warm_sb[:, :], 0.0)
        nc.scalar.activation(warm_sb[:, :], warm_sb[:, :], EXP)  # preload exp table set
        nc.gpsimd.memset(den_sb[:, :], 1.0)
        for kt in range(NT):
            vv = ves_sb[:, kt * VS:(kt + 1) * VS].rearrange("p (h e) -> p h e", e=65)
            nc.gpsimd.memset(vv[:, 0:12, 64:65], 1.0)

        # ---- helpers
        QK_ORD = [0, 6, 1, 7, 2, 8, 3, 9, 4, 10, 5, 11]

        def qk_mtile_half(mt, qb):
            ps = psmall.tile([128, 512], F32, tag="mm", name=f"qk{mt}_{qb}")
            for ct in range(CT):
                nc.tensor.matmul(
                    ps[:, :],
                    lhsT=wqk_sb[:, ct * 1536 + QK_ORD.index(mt) * 128: ct * 1536 + (QK_ORD.index(mt) + 1) * 128],
                    rhs=xT_sb[:, ct * N + qb * 512: ct * N + qb * 512 + 512],
                    start=(ct == 0), stop=(ct == CT - 1),
                )
            nc.vector.tensor_copy(qkT_sb[:, mt * N + qb * 512: mt * N + qb * 512 + 512], ps[:, :])

        def qk_mtile(mt):
            for qb in range(QB):
                qk_mtile_half(mt, qb)

        def v_ntile(nt):
            """Compute V natural rows [nt*128, +128] and scatter into ves (+ones cols)."""
            for vb in range(2):
                ps = pbig.tile([128, 384], F32, tag="big", name=f"v{nt}_{vb}")
                for ct in range(CT):
                    nc.tensor.matmul(
                        ps[:, :],
                        lhsT=xT_sb[:, ct * N + nt * 128: ct * N + (nt + 1) * 128],
                        rhs=wv_sb[:, ct * C + vb * 384: ct * C + (vb + 1) * 384],
                        start=(ct == 0), stop=(ct == CT - 1),
                    )
                dst = ves_sb[:, nt * VS:(nt + 1) * VS].rearrange("p (h e) -> p h e", e=65)
                nc.vector.tensor_copy(
                    dst[:, vb * 6:(vb + 1) * 6, 0:64],
                    ps[:, :].rearrange("p (h e) -> p h e", e=64),
                )

        def q_slice(h, qb):
            po = (h % 2) * 64
            return qkT_sb[po:po + 64, (h // 2) * N + qb * 512: (h // 2) * N + qb * 512 + 512]

        def k_slice(h, kt):
            po = (h % 2) * 64
            base = (6 + h // 2) * N + kt * 128
            return qkT_sb[po:po + 64, base: base + 128]

        # pipeline state
        pt_kt = {}         # (pair, kt, j) -> [128, 1024] bf16 exp tile
        att_tiles = {}     # pair -> [128, 2048] bf16 (rows 0-63 numerators, row 64 denominators)

        def scores_and_exp(p, kt):
            h0, h1 = 2 * p, 2 * p + 1
            ps0 = pbig.tile([128, 1024], F32, tag="big")
            ps1 = pbig.tile([128, 1024], F32, tag="big")
            for qb in range(QB):
                nc.tensor.matmul(ps0[:, qb * 512: qb * 512 + 512], lhsT=k_slice(h0, kt),
                                 rhs=q_slice(h0, qb), start=True, stop=True)
                nc.tensor.matmul(ps1[:, qb * 512: qb * 512 + 512], lhsT=k_slice(h1, kt),
                                 rhs=q_slice(h1, qb), start=True, stop=True)
            pt0 = ptp.tile([128, 1024], BF, tag="pt", name=f"pt{p}_{kt}a")
            pt1 = ptp.tile([128, 1024], BF, tag="pt", name=f"pt{p}_{kt}b")
            pt_kt[(p, kt, 0)], pt_kt[(p, kt, 1)] = pt0, pt1
            nc.scalar.activation(pt0[:, :], ps0[:, :], EXP)
            nc.scalar.activation(pt1[:, :], ps1[:, :], EXP)

        po_open = {}

        def attn_burst_half(p, idx):
            """Half of an attn@V accumulation group (4 MMs); idx 0..7 walks
            (h0,qb0),(h0,qb1),(h1,qb0),(h1,qb1) two slots each. Group closes
            and evacuates on the odd idx."""
            g = idx // 2
            j, qb = g // 2, g % 2
            h = 2 * p + j
            if idx % 2 == 0:
                po_open[(p, j, qb)] = psmall.tile([65, 512], F32, tag="mm", name=f"po{h}_{qb}")
            po = po_open[(p, j, qb)]
            k0 = (idx % 2) * 4
            for kt in range(k0, k0 + 4):
                nc.tensor.matmul(
                    po[:, :],
                    lhsT=ves_sb[:, kt * VS + h * 65: kt * VS + h * 65 + 65],
                    rhs=pt_kt[(p, kt, j)][:, qb * 512: qb * 512 + 512],
                    start=(kt == 0), stop=(kt == NT - 1),
                )
            if idx % 2 == 1:
                po_open.pop((p, j, qb))
                if p not in att_tiles:
                    att_tiles[p] = attp.tile([128, 2 * N], BF, tag="att", name=f"att{p}")
                att_t = att_tiles[p]
                nc.vector.tensor_copy(
                    att_t[0:65, j * 1024 + qb * 512: j * 1024 + qb * 512 + 512], po[:, :])
                if qb == 1:
                    for kt in range(NT):
                        pt_kt.pop((p, kt, j))
                    nc.sync.dma_start(out=den_sb[h:h + 1, :], in_=att_t[64:65, j * 1024: j * 1024 + 1024])

        def attn_burst(p, j, qb):
            attn_burst_half(p, (j * 2 + qb) * 2)
            attn_burst_half(p, (j * 2 + qb) * 2 + 1)

        def recip_pair(p):
            # full-tile ops: partition bases other than 0/32/64/96 are illegal,
            # so recompute all 12 rows (unwritten rows hold memset 1.0)
            nc.vector.tensor_copy(denf_sb[:, :], den_sb[:, :])
            nc.vector.reciprocal_approx_fast(recip_sb[:, :], denf_sb[:, :])
            nc.vector.tensor_copy(rb_sb[:, :], recip_sb[:, :])

        def divide_head(p, j, pool_tag="mm"):
            att_t = att_tiles[p]
            h = 2 * p + j
            po = (h % 2) * 64
            for qb in range(QB):
                pool = psmall if pool_tag == "mm" else pbig
                pb = pool.tile([64, 512], F32, tag=pool_tag, name=f"pb{h}_{qb}")
                nc.tensor.matmul(
                    pb[:, :],
                    lhsT=sel_sb[0:12, h * 64:(h + 1) * 64],
                    rhs=rb_sb[0:12, qb * 512: qb * 512 + 512],
                    start=True, stop=True)
                nc.vector.tensor_mul(
                    attf_sb[po:po + 64, (h // 2) * N + qb * 512: (h // 2) * N + qb * 512 + 512],
                    att_t[0:64, j * 1024 + qb * 512: j * 1024 + qb * 512 + 512],
                    pb[:, :])
            if j == 1:
                att_tiles.pop(p)

        # ---- emission schedule: scores/exp of pair p overlap attn@V bursts,
        # evac, recip and divide of pair p-1, plus qkT tiles for pair p+1
        qk_mtile_half(0, 0); qk_mtile_half(6, 0)
        qk_mtile_half(0, 1); qk_mtile_half(6, 1)

        for p in range(PAIRS):
            for kt in range(NT):
                scores_and_exp(p, kt)
                if p == 0:
                    v_ntile(kt)
                if p >= 1:
                    if kt < 4:
                        attn_burst(p - 1, kt // 2, kt % 2)
                    elif kt == 4:
                        recip_pair(p - 1)
                    elif kt in (5, 6):
                        divide_head(p - 1, kt - 5)
                if p + 1 < PAIRS and 2 <= kt < 6:
                    # order (q,qb0),(q,qb1),(k,qb0),(k,qb1): S(p+1,kt0) needs the
                    # first three, so its gating evac lands a slot earlier
                    mt, qb = [(p + 1, 0), (p + 1, 1), (p + 7, 0), (p + 7, 1)][kt - 2]
                    qk_mtile_half(mt, qb)

        # ---- drain: open projection groups on ct0-4 around the last pair's
        # attn@V and divide chain so the PE stays busy (and warm) throughout
        open_groups = {}

        def open_proj(nt, mb, pool, tag):
            ps = pool.tile([128, 384], F32, tag=tag, name=f"y{nt}_{mb}")
            for ct in range(CT - 1):
                nc.tensor.matmul(
                    ps[:, :],
                    lhsT=attf_sb[:, ct * N + nt * 128: ct * N + (nt + 1) * 128],
                    rhs=wp_sb[:, ct * C + mb * 384: ct * C + (mb + 1) * 384],
                    start=(ct == 0), stop=False,
                )
            open_groups[(nt, mb)] = ps

        open_proj(0, 0, pbig, "big")
        open_proj(0, 1, pbig, "big")
        for k4 in range(4):
            attn_burst(PAIRS - 1, k4 // 2, k4 % 2)
        open_proj(1, 0, psmall, "mm")
        open_proj(1, 1, psmall, "mm")
        recip_pair(PAIRS - 1)
        divide_head(PAIRS - 1, 0, pool_tag="big")
        divide_head(PAIRS - 1, 1, pool_tag="big")

        # ---- output projection: y[n, m] = attf.T @ wpT (+ bias on host)
        for nt in range(NT):
            y_t = yp.tile([128, C], F32, tag="y")
            for mb in range(2):
                if (nt, mb) in open_groups:
                    ps = open_groups[(nt, mb)]
                    ct = CT - 1
                    nc.tensor.matmul(
                        ps[:, :],
                        lhsT=attf_sb[:, ct * N + nt * 128: ct * N + (nt + 1) * 128],
                        rhs=wp_sb[:, ct * C + mb * 384: ct * C + (mb + 1) * 384],
                        start=False, stop=True,
                    )
                else:
                    ps = psmall.tile([128, 384], F32, tag="mm", name=f"y{nt}_{mb}")
                    for ct in range(CT):
                        nc.tensor.matmul(
                            ps[:, :],
                            lhsT=attf_sb[:, ct * N + nt * 128: ct * N + (nt + 1) * 128],
                            rhs=wp_sb[:, ct * C + mb * 384: ct * C + (mb + 1) * 384],
                            start=(ct == 0), stop=(ct == CT - 1),
                        )
                if mb == 0:
                    nc.scalar.copy(y_t[:, mb * 384:(mb + 1) * 384], ps[:, :])
                else:
                    nc.vector.tensor_copy(y_t[:, mb * 384:(mb + 1) * 384], ps[:, :])
                nc.sync.dma_start(out=out[nt * 128:(nt + 1) * 128, mb * 384:(mb + 1) * 384],
                                  in_=y_t[:, mb * 384:(mb + 1) * 384])

    nc.compile()
    return nc


_CACHE = {}


def _prep_inputs(x, w_qkv, w_proj):
    bf = ml_dtypes.bfloat16
    w = np.array(w_qkv, dtype=np.float32, copy=True)
    w[:C] *= SCALE
    wqkT = w[:2 * C].T.astype(bf)                                # [C, 2C]
    ord_ = [0, 6, 1, 7, 2, 8, 3, 9, 4, 10, 5, 11]
    wqkA = np.ascontiguousarray(np.concatenate([wqkT[:, mt * 128:(mt + 1) * 128] for mt in ord_[:4]], axis=1))
    wqkB = np.ascontiguousarray(np.concatenate([wqkT[:, mt * 128:(mt + 1) * 128] for mt in ord_[4:]], axis=1))
    wvT = np.ascontiguousarray(w[2 * C:].T.astype(bf))          # [C, C]
    wpT = np.ascontiguousarray(np.asarray(w_proj).T.astype(bf))  # [C, C]
    sel = np.zeros((12, H * 64), dtype=bf)
    for h in range(H):
        sel[h, h * 64:(h + 1) * 64] = 1.0
    maps = []
    for b in range(B):
        maps.append({
            "xT": np.ascontiguousarray(np.asarray(x[b]).T.astype(bf)),
            "wqkA": wqkA, "wqkB": wqkB, "wvT": wvT, "wpT": wpT, "sel": sel,
        })
    return maps


def kernel(x, w_qkv, w_proj, b_proj):
    from concourse.bass_utils import run_bass_kernel_spmd

    if "nc" not in _CACHE:
        _CACHE["nc"] = build_nc()
    nc = _CACHE["nc"]
    in_maps = _prep_inputs(x, w_qkv, w_proj)
    res = run_bass_kernel_spmd(nc, in_maps, core_ids=list(range(B)))
    y = np.stack([np.asarray(res.results[i]["out"], dtype=np.float32) for i in range(B)])
    y = y + np.asarray(b_proj, dtype=np.float32)[None, None, :]
    return y.astype(np.float32)


if __name__ == "__main__":
    nc = build_nc()
    print("build OK")

